# revision 34
# baseline (speedup 1.0000x reference)
"""Multi-head attention Trainium2 Bass kernel (fp8-DoubleRow version).

Problem: B=8, N=2048, C=768, H=12 heads, D=64 head dim.
  qkv = x @ w_qkv.T          -> [B, N, 3C]
  per head: softmax(q k^T / sqrt(D)) @ v
  y = attn_out @ w_proj.T + b_proj

Sharding: data parallel over batch - one batch element per NeuronCore.

Mixed-precision strategy (cost model: matmul = out_free x pe_cycle x cpr,
fp8 DoubleRow cpr=0.5 vs fp32r 1.0):
  - QKV: fp32r (exact q/k/v; fp8 here costs too much accuracy).
  - S = k^T q: fp8 DoubleRow. lhsT = k8 duplicated via a stride-0 block dim,
    rhs blocks = (q_hi, q_lo) -> S = k8^T (q_hi + q_lo): q at ~14-bit
    precision, k at fp8, half the fp32r PE cost.
  - exp: ~11.75/16 kv-tiles on ScalarE (activation Exp -> fp8 out), the
    rest via a Schraudolph fast-exp: DVE affine to int16 (bf16 exponent
    bits), gpsimd converts bf16 -> fp8e4m3. Both heads of a schr tile
    share one 2-bank psum so the affine is a single [128,2,512] op; one
    extra tile alternates engines per chunk to balance ScalarE/DVE.
  - AV: fp8 DoubleRow pairing kv-tiles. lhsT M-dim = [v_hi d0-63 | ones |
    v_lo d1-63]: v at ~14-bit, denominator row free (psum row 64).
  - normalize: one reciprocal of the denominator row + partition_broadcast
    + one full-128-row multiply (partition count is free in the cost
    model). The hi+lo recombination is deferred to the projection, which
    contracts 1536 channels against host-duplicated w_proj rows; the
    denominator row (denom*recip == 1) hits a zeroed weight row.
  - proj: fp32r over the extended 1536-channel aT, tail-only with a wide
    psum rotation over the freed attention banks.

Scheduling: engines execute their streams in order, so emission order is
the schedule. QKV chunks of pair hp+1 are emitted after each q-chunk of
pair hp's attention (j-interleave); the Schraudolph groups sit on their
own psum ring so the DVE never gates the ScalarE act ring.
"""

import numpy as np

import concourse.bass as bass
import concourse.mybir as mybir
import concourse.tile as tile
from concourse import bacc
from concourse.bass_utils import run_bass_kernel_spmd
from concourse.masks import make_identity

B, N, C, H = 8, 2048, 768, 12
D = C // H            # 64
F = 3 * C             # 2304
NT = N // 128         # 16 seq tiles
CT = C // 128         # 6 channel tiles
NQ = 512              # query-chunk width (1 psum bank of fp32)
NCH = N // NQ         # 4 chunks
SCALE = float(D) ** -0.5

FP32 = mybir.dt.float32
FP32R = mybir.dt.float32r
FP8 = mybir.dt.float8e4
I16 = mybir.dt.int16
BF16 = mybir.dt.bfloat16
EXP = mybir.ActivationFunctionType.Exp
DR = mybir.MatmulPerfMode.DoubleRow
ADD = mybir.AluOpType.add
SUB = mybir.AluOpType.subtract
MULT = mybir.AluOpType.mult

# Schraudolph constants: bf16 bits = 128*log2(exp(s_eff)) + 127*128,
# s_eff = S_psum * SCALE -> bits = S * (128*SCALE/ln2) + 16256.
SCHR_A = 128.0 * SCALE / float(np.log(2.0))
SCHR_B = 16256.0
N_SCHR_GROUPS = 2      # of 8 groups of 2 kv-tiles: last N on DVE+Pool

_CACHED_NC = None


def _dup2(ap):
    """Insert a stride-0 [0,2] block dim after the partition dim."""
    return bass.AP(
        tensor=ap.tensor,
        offset=ap.offset,
        ap=[list(ap.ap[0]), [0, 2]] + [list(d) for d in ap.ap[1:]],
    )


def _bc_ap(dram_ap, parts):
    """Partition-broadcast a 1-D DRAM AP to [parts, len] via stride-0."""
    return bass.AP(
        tensor=dram_ap.tensor,
        offset=dram_ap.offset,
        ap=[[0, parts]] + [list(p) for p in dram_ap.ap],
    )


def build():
    nc = bacc.Bacc()
    x = nc.dram_tensor("xT", [C, N], FP32, kind="ExternalInput")
    w_qkv = nc.dram_tensor("w_qkvT", [C, F], FP32, kind="ExternalInput")
    w_proj = nc.dram_tensor("w_projT", [2 * C, C], FP32, kind="ExternalInput")
    b_proj = nc.dram_tensor("b_proj", [C], FP32, kind="ExternalInput")
    y = nc.dram_tensor("y", [N, C], FP32, kind="ExternalOutput")
    C2 = 2 * C
    aT_d = nc.dram_tensor("aT_scratch", [C2, N], FP32R)

    xr = x[:, :].bitcast(FP32R)
    wqr = w_qkv[:, :].bitcast(FP32R)
    wpr = w_proj[:, :].bitcast(FP32R)

    lp = nc.allow_low_precision("fp8 attention with hi/lo compensation")
    lp.__enter__()
    with tile.TileContext(nc) as tc:
        const_cm = tc.tile_pool(name="const", bufs=1)
        const = const_cm.__enter__()
        ident_f = const.tile([128, 128], FP32)
        make_identity(nc, ident_f)
        ident = const.tile([128, 128], FP32R)
        nc.vector.tensor_copy(ident, ident_f)
        xr3 = xr.rearrange("(ko p) n -> p ko n", p=128)
        wqr3 = wqr.rearrange("(ko p) f -> p ko f", p=128)

        with tc.tile_pool(name="hpool", bufs=2) as hpool, \
             tc.tile_pool(name="spool", bufs=1) as spool, \
             tc.tile_pool(name="small", bufs=2) as small, \
             tc.tile_pool(name="psum_s", bufs=2, space="PSUM") as psum_s, \
             tc.tile_pool(name="psum_av", bufs=2, space="PSUM") as psum_av, \
             tc.tile_pool(name="psum_qkv", bufs=1, space="PSUM") as psum_qkv:

            class QkvPhase:
                """QKV chunks for one head pair, emitted j-at-a-time so the
                matmuls/copies interleave with the previous pair's
                attention on every engine stream."""

                def __init__(self, hp, rot=("qkvps",)):
                    self.hp = hp
                    self.rot = rot
                    self.xc0 = None
                    if hp == 0:
                        # prefetch the first x chunk ahead of the wq loads
                        # so the first qkv matmul chain starts ~5us sooner
                        self.xc0 = hpool.tile(
                            [128, CT, NQ], FP32R, tag="xc", name="xc",
                        )
                        nc.sync.dma_start(out=self.xc0, in_=xr3[:, :, 0:NQ])
                    self.wq = hpool.tile(
                        [128, CT, 3, 128], FP32R, tag="wq", name="wq", bufs=1,
                    )
                    for idx, m in enumerate((hp, CT + hp, 2 * CT + hp)):
                        nc.sync.dma_start(
                            out=self.wq[:, :, idx, :],
                            in_=wqr3[:, :, m * 128:(m + 1) * 128],
                        )
                    self.qTt = hpool.tile([128, 2, N], FP8, tag="qT")
                    self.kTt = hpool.tile([128, N], FP8, tag="kT")
                    self.vTt = hpool.tile([128, N], FP32R, tag="vT", bufs=1)

                def emit_j(self, j):
                    if j == 0 and self.xc0 is not None:
                        xc = self.xc0
                    else:
                        xc = hpool.tile(
                            [128, CT, NQ], FP32R, tag="xc", name="xc",
                        )
                        nc.sync.dma_start(
                            out=xc, in_=xr3[:, :, j * NQ:(j + 1) * NQ]
                        )
                    js = slice(j * NQ, (j + 1) * NQ)
                    for idx in range(3):
                        tg = self.rot[idx % len(self.rot)]
                        if tg == "qkvps":
                            ps = psum_qkv.tile(
                                [128, NQ], FP32, tag="qkvps", name="qkvps"
                            )
                        else:
                            ps = psum_s.tile(
                                [128, NQ], FP32, tag="spsX", name="spsX",
                                bufs=1,
                            )
                        for k in range(CT):
                            nc.tensor.matmul(
                                ps,
                                self.wq[:, k, idx, :],
                                xc[:, k, :],
                                start=(k == 0),
                                stop=(k == CT - 1),
                            )
                        if idx == 0:      # q -> hi + lo fp8
                            nc.vector.tensor_copy(self.qTt[:, 0, js], ps)
                            nc.vector.tensor_tensor(
                                self.qTt[:, 1, js], ps, self.qTt[:, 0, js],
                                SUB,
                            )
                        elif idx == 1:    # k -> fp8
                            nc.vector.tensor_copy(self.kTt[:, js], ps)
                        else:             # v -> fp32r (transposed later)
                            nc.vector.tensor_copy(self.vTt[:, js], ps)

                def finish(self):
                    # v transposes; vaug = [v_hi d0-63 | ones | v_lo d1-63]
                    vaugs = []
                    for a in range(2):
                        vaug = hpool.tile([128, NT, 128], FP8, tag=f"vaug{a}")
                        nc.vector.memset(vaug[:, :, D:D + 1], 1.0)
                        vaugs.append(vaug)
                    for t0 in range(0, NT, 8):
                        pts = [
                            psum_av.tile(
                                [128, 8, D], FP32R, tag="av", name=f"pt{a}",
                                bufs=1,
                            )
                            for a in range(2)
                        ]
                        for g in range(8):
                            t = t0 + g
                            for a in range(2):
                                lo = a * D
                                nc.tensor.transpose(
                                    pts[a][:, g, :],
                                    self.vTt[lo:lo + D,
                                             t * 128:(t + 1) * 128],
                                    ident[lo:lo + D, lo:lo + D],
                                )
                        for a in range(2):
                            nc.vector.tensor_copy(
                                vaugs[a][:, t0:t0 + 8, 0:D], pts[a]
                            )
                            nc.vector.tensor_tensor(
                                vaugs[a][:, t0:t0 + 8, D + 1:128],
                                pts[a][:, :, 1:D],
                                vaugs[a][:, t0:t0 + 8, 1:D],
                                SUB,
                            )
                    return self.qTt, self.kTt, vaugs

            def emit_attention(hp, qTt, kTt, vaugs, qkv_cb=None, post_j=None):
                # ---- attention per q-chunk. aT tiles hold the full 128-row
                # [hi | denom*r | lo] block; proj contracts the doubled
                # channel dim with host-duplicated w_proj rows.
                aTts = [
                    hpool.tile([128, N], FP32, tag=f"aTt{a}", name=f"aTt{a}")
                    for a in range(2)
                ]
                for j in range(NCH):
                    js = slice(j * NQ, (j + 1) * NQ)
                    expSs = [
                        spool.tile(
                            [128, NT, NQ], FP8,
                            tag=f"expS{a}", name=f"expS{a}", bufs=2,
                        )
                        for a in range(2)
                    ]
                    def s_mm(out_ap, a, t):
                        lo = a * D
                        kap = kTt[lo:lo + D, t * 128:(t + 1) * 128]
                        nc.tensor.matmul(
                            out_ap,
                            _dup2(kap),
                            qTt[lo:lo + D, :, js],
                            start=True,
                            stop=True,
                            perf_mode=DR,
                        )

                    def emit_act_group(g, heads=(0, 1)):
                        # tiles (2g, 2g+1) -> ScalarE exp, own sps0/1 ring
                        t = 2 * g
                        sps_ab = {
                            a: psum_s.tile(
                                [128, 2, NQ], FP32, tag=f"sps{a}",
                                name=f"sps{a}", bufs=1,
                            )
                            for a in heads
                        }
                        for u in range(2):
                            for a in heads:
                                s_mm(sps_ab[a][:, u, :], a, t + u)
                        for a in heads:
                            nc.scalar.activation(
                                out=expSs[a][:, t:t + 2, :],
                                in_=sps_ab[a][:, :, :],
                                func=EXP,
                                scale=SCALE,
                            )

                    def emit_schr_tile(t, heads=(0, 1)):
                        # kv-tile on the DVE schraudolph ring. When both
                        # heads run, their S psums share one 2-bank spsX
                        # tile so the DVE affine is a single [128,2,512] op.
                        spsx = psum_s.tile(
                            [128, 2, NQ], FP32, tag="spsX",
                            name="spsX", bufs=1,
                        )
                        for i, a in enumerate(heads):
                            s_mm(spsx[:, i, :], a, t)
                        i16 = small.tile(
                            [128, 2, NQ], I16, tag="i16", name="i16",
                        )
                        n_h = len(heads)
                        nc.vector.tensor_scalar(
                            i16[:, 0:n_h, :], spsx[:, 0:n_h, :],
                            SCHR_A, SCHR_B, MULT, ADD,
                        )
                        for i, a in enumerate(heads):
                            nc.gpsimd.tensor_copy(
                                expSs[a][:, t:t + 1, :],
                                i16[:, i, :].bitcast(BF16),
                            )

                    def emit_act_single(t, a):
                        # one kv-tile, one head on ScalarE
                        sps = psum_s.tile(
                            [128, 2, NQ], FP32, tag=f"sps{a}",
                            name=f"sps{a}", bufs=1,
                        )
                        s_mm(sps[:, 0, :], a, t)
                        nc.scalar.activation(
                            out=expSs[a][:, t:t + 1, :],
                            in_=sps[:, 0:1, :],
                            func=EXP,
                            scale=SCALE,
                        )

                    # head0: act tiles 0-9 + single 10, schr 11-15;
                    # head1: act tiles 0-11, schr 12-15. Tiles 12-15 run
                    # both heads in one paired DVE affine; the asymmetric
                    # split moves half a tile of exp off the ScalarE pacer.
                    emit_act_group(0)
                    emit_schr_tile(12)
                    emit_act_group(1)
                    emit_schr_tile(13)
                    emit_act_group(2)
                    odd = (hp + j) % 2 == 1
                    if odd:
                        emit_schr_tile(11, heads=(0,))
                    emit_act_group(3)
                    emit_schr_tile(14)
                    emit_act_group(4)
                    emit_schr_tile(15)
                    if odd:
                        emit_act_group(5, heads=(1,))
                        emit_act_single(10, 0)
                    else:
                        emit_act_group(5)
                    avs = []
                    for a in range(2):
                        av = psum_av.tile([128, NQ], FP32, tag="av", bufs=1)
                        for u in range(NT // 2):
                            nc.tensor.matmul(
                                av,
                                vaugs[a][:, 2 * u:2 * u + 2, :],
                                expSs[a][:, 2 * u:2 * u + 2, :],
                                start=(u == 0),
                                stop=(u == NT // 2 - 1),
                                perf_mode=DR,
                            )
                        avs.append(av)
                    # next pair's qkv copies go ahead of the normalize in
                    # the DVE stream: they only wait on quick qkv matmuls,
                    # while the normalize waits on the whole AV chain
                    if qkv_cb is not None:
                        qkv_cb(j)
                    for a in range(2):
                        av = avs[a]
                        # normalize all 128 rows at once (partition count is
                        # free); row 64 becomes denom*recip = 1 and hits a
                        # zero row of the extended w_proj
                        recip = small.tile([1, NQ], FP32, tag="recip")
                        nc.vector.reciprocal(recip, av[D:D + 1, :])
                        bc128 = small.tile([128, NQ], FP32, tag="bc128")
                        nc.gpsimd.partition_broadcast(bc128, recip)
                        nc.vector.tensor_tensor(
                            aTts[a][:, js], av, bc128, MULT
                        )
                    # half-pair aT stores let the proj overlap the tail of
                    # the last pair's attention
                    if j % 2 == 1:
                        js2 = slice((j - 1) * NQ, (j + 1) * NQ)
                        for a in range(2):
                            h = 2 * hp + a
                            nc.sync.dma_start(
                                out=aT_d[h * 128:(h + 1) * 128, js2].bitcast(
                                    FP32
                                ),
                                in_=aTts[a][:, js2],
                            )
                    if post_j is not None:
                        post_j(j)

            # proj weights/bias DMA'd up front on the gpsimd DMA queue so
            # they don't delay the first pair's x/wq loads on SP
            bias_bc = small.tile([128, C], FP32, tag="bias", bufs=1)
            nc.gpsimd.dma_start(out=bias_bc, in_=_bc_ap(b_proj[:], 128))
            w_projT = small.tile([128, 2 * CT, C], FP32R, tag="wproj",
                                 bufs=1)
            nc.gpsimd.dma_start(
                out=w_projT, in_=wpr.rearrange("(ko p) o -> p ko o", p=128)
            )

            NO = 384

            def emit_proj(i2_list, psum_tags):
                for n_p, i2 in enumerate(i2_list):
                    a_sb = small.tile(
                        [128, 2 * CT, 256], FP32R, tag="a_sb", bufs=2
                    )
                    nc.sync.dma_start(
                        out=a_sb,
                        in_=aT_d[:, i2 * 256:(i2 + 1) * 256].rearrange(
                            "(ko p) n -> p ko n", p=128
                        ),
                    )
                    for ii in range(2):
                        i = 2 * i2 + ii
                        y_sb = small.tile([128, C], FP32, tag="y_sb", bufs=2)
                        for half in range(2):
                            tg = psum_tags[(n_p * 4 + ii * 2 + half)
                                           % len(psum_tags)]
                            if tg == "qkvps":
                                psf = psum_qkv.tile(
                                    [128, NQ], FP32, tag="qkvps",
                                    name="qkvps")
                            elif tg == "spsX":
                                psf = psum_s.tile(
                                    [128, NQ], FP32, tag="spsX",
                                    name="spsX", bufs=1)
                            elif tg in ("sps0", "sps1"):
                                psf = psum_s.tile(
                                    [128, 2, NQ], FP32, tag=tg,
                                    name=tg, bufs=1)[:, 0, :]
                            else:
                                psf = psum_av.tile(
                                    [128, NQ], FP32, tag="av", bufs=1)
                            ps = psf[:, 0:NO]
                            for k in range(2 * CT):
                                nc.tensor.matmul(
                                    ps,
                                    a_sb[:, k, ii * 128:(ii + 1) * 128],
                                    w_projT[:, k, half * NO:(half + 1) * NO],
                                    start=(k == 0),
                                    stop=(k == CT - 1),
                                )
                            nc.vector.tensor_add(
                                y_sb[:, half * NO:(half + 1) * NO], ps,
                                bias_bc[:, half * NO:(half + 1) * NO]
                            )
                        nc.sync.dma_start(
                            out=y[i * 128:(i + 1) * 128, :],
                            in_=y_sb,
                        )

            def last_pair_post_j(j):
                # overlap most of the projection with the last pair's
                # attention, using the qkv psum bank plus the freed sps1 ring
                pass

            cur = QkvPhase(0, rot=("qkvps", "spsX"))
            for j in range(NCH):
                cur.emit_j(j)
            tiles = cur.finish()
            for hp in range(H // 2):
                nxt = QkvPhase(hp + 1) if hp + 1 < H // 2 else None
                emit_attention(
                    hp, *tiles,
                    qkv_cb=nxt.emit_j if nxt is not None else None,
                    post_j=last_pair_post_j if hp == H // 2 - 1 else None,
                )
                if nxt is not None:
                    tiles = nxt.finish()

            # ---------- rest of the projection; attention psums now free
            emit_proj([0, 1, 2, 3, 4, 5, 6, 7],
                      ["qkvps", "sps0", "sps1", "spsX", "av"])
        const_cm.__exit__(None, None, None)
    lp.__exit__(None, None, None)

    nc.finalize()
    return nc


def get_nc():
    global _CACHED_NC
    if _CACHED_NC is None:
        _CACHED_NC = build()
    return _CACHED_NC


LAST_RESULT = None


def kernel(x, w_qkv, w_proj, b_proj, **run_kwargs):
    x = np.ascontiguousarray(np.asarray(x, dtype=np.float32))
    w_qkv = np.ascontiguousarray(np.asarray(w_qkv, dtype=np.float32))
    w_proj = np.ascontiguousarray(np.asarray(w_proj, dtype=np.float32))
    b_proj = np.ascontiguousarray(np.asarray(b_proj, dtype=np.float32))
    assert x.shape == (B, N, C)

    nc = get_nc()
    w_qkvT = np.ascontiguousarray(w_qkv.T)
    # extended proj weights: per head the aT block is
    # [hi d0-63 | denom*r (==1) | v_lo d1-63]; duplicate w rows for the lo
    # part, zero the denom row.
    w_ext = np.zeros((2 * C, C), dtype=np.float32)
    for h in range(H):
        blk = w_proj[:, h * D:(h + 1) * D]          # [C_out, 64]
        w_ext[h * 128:h * 128 + 64, :] = blk.T
        w_ext[h * 128 + 65:h * 128 + 128, :] = blk.T[1:64]
    w_projT = np.ascontiguousarray(w_ext)
    b_eff = np.ascontiguousarray(b_proj.astype(np.float32))
    in_maps = [
        {
            "xT": np.ascontiguousarray(x[i].T),
            "w_qkvT": w_qkvT,
            "w_projT": w_projT,
            "b_proj": b_eff,
        }
        for i in range(B)
    ]
    res = run_bass_kernel_spmd(nc, in_maps, list(range(B)), **run_kwargs)
    global LAST_RESULT
    LAST_RESULT = res
    out = np.stack([res.results[i]["y"] for i in range(B)], axis=0)
    return out


if __name__ == "__main__":
    rng = np.random.default_rng(0)
    x = rng.standard_normal((B, N, C), dtype=np.float32)
    w_qkv = (rng.standard_normal((F, C)) * 0.02).astype(np.float32)
    w_proj = (rng.standard_normal((C, C)) * 0.02).astype(np.float32)
    b_proj = (rng.standard_normal((C,)) * 0.02).astype(np.float32)
    out = kernel(x=x, w_qkv=w_qkv, w_proj=w_proj, b_proj=b_proj)
    print("out", out.shape, out.dtype, float(np.abs(out).max()))


# revision 35
# speedup vs baseline: 1.0039x; 1.0039x over previous
"""Multi-head attention Trainium2 Bass kernel (fp8-DoubleRow version).

Problem: B=8, N=2048, C=768, H=12 heads, D=64 head dim.
  qkv = x @ w_qkv.T          -> [B, N, 3C]
  per head: softmax(q k^T / sqrt(D)) @ v
  y = attn_out @ w_proj.T + b_proj

Sharding: data parallel over batch - one batch element per NeuronCore.

Mixed-precision strategy (cost model: matmul = out_free x pe_cycle x cpr,
fp8 DoubleRow cpr=0.5 vs fp32r 1.0):
  - QKV: fp32r (exact q/k/v; fp8 here costs too much accuracy).
  - S = k^T q: fp8 DoubleRow. lhsT = k8 duplicated via a stride-0 block dim,
    rhs blocks = (q_hi, q_lo) -> S = k8^T (q_hi + q_lo): q at ~14-bit
    precision, k at fp8, half the fp32r PE cost.
  - exp: ~11.75/16 kv-tiles on ScalarE (activation Exp -> fp8 out), the
    rest via a Schraudolph fast-exp: DVE affine to int16 (bf16 exponent
    bits), gpsimd converts bf16 -> fp8e4m3. Both heads of a schr tile
    share one 2-bank psum so the affine is a single [128,2,512] op; one
    extra tile alternates engines per chunk to balance ScalarE/DVE.
  - AV: fp8 DoubleRow pairing kv-tiles. lhsT M-dim = [v_hi d0-63 | ones |
    v_lo d1-63]: v at ~14-bit, denominator row free (psum row 64).
  - normalize: one reciprocal of the denominator row + partition_broadcast
    + one full-128-row multiply (partition count is free in the cost
    model). The hi+lo recombination is deferred to the projection, which
    contracts 1536 channels against host-duplicated w_proj rows; the
    denominator row (denom*recip == 1) hits a zeroed weight row.
  - proj: fp32r over the extended 1536-channel aT, tail-only with a wide
    psum rotation over the freed attention banks.

Scheduling: engines execute their streams in order, so emission order is
the schedule. QKV chunks of pair hp+1 are emitted after each q-chunk of
pair hp's attention (j-interleave); the Schraudolph groups sit on their
own psum ring so the DVE never gates the ScalarE act ring.
"""

import numpy as np

import concourse.bass as bass
import concourse.mybir as mybir
import concourse.tile as tile
from concourse import bacc
from concourse.bass_utils import run_bass_kernel_spmd
from concourse.masks import make_identity

B, N, C, H = 8, 2048, 768, 12
D = C // H            # 64
F = 3 * C             # 2304
NT = N // 128         # 16 seq tiles
CT = C // 128         # 6 channel tiles
NQ = 512              # query-chunk width (1 psum bank of fp32)
NCH = N // NQ         # 4 chunks
SCALE = float(D) ** -0.5

FP32 = mybir.dt.float32
FP32R = mybir.dt.float32r
FP8 = mybir.dt.float8e4
I16 = mybir.dt.int16
BF16 = mybir.dt.bfloat16
EXP = mybir.ActivationFunctionType.Exp
DR = mybir.MatmulPerfMode.DoubleRow
ADD = mybir.AluOpType.add
SUB = mybir.AluOpType.subtract
MULT = mybir.AluOpType.mult

# Schraudolph constants: bf16 bits = 128*log2(exp(s_eff)) + 127*128,
# s_eff = S_psum * SCALE -> bits = S * (128*SCALE/ln2) + 16256.
SCHR_A = 128.0 * SCALE / float(np.log(2.0))
SCHR_B = 16256.0
N_SCHR_GROUPS = 2      # of 8 groups of 2 kv-tiles: last N on DVE+Pool

_CACHED_NC = None


def _dup2(ap):
    """Insert a stride-0 [0,2] block dim after the partition dim."""
    return bass.AP(
        tensor=ap.tensor,
        offset=ap.offset,
        ap=[list(ap.ap[0]), [0, 2]] + [list(d) for d in ap.ap[1:]],
    )


def _bc_ap(dram_ap, parts):
    """Partition-broadcast a 1-D DRAM AP to [parts, len] via stride-0."""
    return bass.AP(
        tensor=dram_ap.tensor,
        offset=dram_ap.offset,
        ap=[[0, parts]] + [list(p) for p in dram_ap.ap],
    )


def build():
    nc = bacc.Bacc()
    x = nc.dram_tensor("xT", [C, N], FP32, kind="ExternalInput")
    w_qkv = nc.dram_tensor("w_qkvT", [C, F], FP32, kind="ExternalInput")
    w_proj = nc.dram_tensor("w_projT", [2 * C, C], FP32, kind="ExternalInput")
    b_proj = nc.dram_tensor("b_proj", [C], FP32, kind="ExternalInput")
    y = nc.dram_tensor("y", [N, C], FP32, kind="ExternalOutput")
    C2 = 2 * C
    aT_d = nc.dram_tensor("aT_scratch", [C2, N], FP32R)

    xr = x[:, :].bitcast(FP32R)
    wqr = w_qkv[:, :].bitcast(FP32R)
    wpr = w_proj[:, :].bitcast(FP32R)

    lp = nc.allow_low_precision("fp8 attention with hi/lo compensation")
    lp.__enter__()
    with tile.TileContext(nc) as tc:
        const_cm = tc.tile_pool(name="const", bufs=1)
        const = const_cm.__enter__()
        ident_f = const.tile([128, 128], FP32)
        make_identity(nc, ident_f)
        ident = const.tile([128, 128], FP32R)
        nc.vector.tensor_copy(ident, ident_f)
        xr3 = xr.rearrange("(ko p) n -> p ko n", p=128)
        wqr3 = wqr.rearrange("(ko p) f -> p ko f", p=128)

        with tc.tile_pool(name="hpool", bufs=2) as hpool, \
             tc.tile_pool(name="spool", bufs=1) as spool, \
             tc.tile_pool(name="small", bufs=2) as small, \
             tc.tile_pool(name="psum_s", bufs=2, space="PSUM") as psum_s, \
             tc.tile_pool(name="psum_av", bufs=2, space="PSUM") as psum_av, \
             tc.tile_pool(name="psum_qkv", bufs=1, space="PSUM") as psum_qkv:

            class QkvPhase:
                """QKV chunks for one head pair, emitted j-at-a-time so the
                matmuls/copies interleave with the previous pair's
                attention on every engine stream."""

                def __init__(self, hp, rot=("qkvps",)):
                    self.hp = hp
                    self.rot = rot
                    self.xc0 = None
                    if hp == 0:
                        # prefetch the first x chunk ahead of the wq loads
                        # so the first qkv matmul chain starts ~5us sooner
                        self.xc0 = hpool.tile(
                            [128, CT, NQ], FP32R, tag="xc", name="xc",
                        )
                        nc.sync.dma_start(out=self.xc0, in_=xr3[:, :, 0:NQ])
                    self.wq = hpool.tile(
                        [128, CT, 3, 128], FP32R, tag="wq", name="wq", bufs=1,
                    )
                    for idx, m in enumerate((hp, CT + hp, 2 * CT + hp)):
                        nc.sync.dma_start(
                            out=self.wq[:, :, idx, :],
                            in_=wqr3[:, :, m * 128:(m + 1) * 128],
                        )
                    self.qTt = hpool.tile([128, 2, N], FP8, tag="qT")
                    self.kTt = hpool.tile([128, N], FP8, tag="kT")
                    self.vTt = hpool.tile([128, N], FP32R, tag="vT", bufs=1)

                def emit_j(self, j):
                    if j == 0 and self.xc0 is not None:
                        xc = self.xc0
                    else:
                        xc = hpool.tile(
                            [128, CT, NQ], FP32R, tag="xc", name="xc",
                        )
                        nc.sync.dma_start(
                            out=xc, in_=xr3[:, :, j * NQ:(j + 1) * NQ]
                        )
                    js = slice(j * NQ, (j + 1) * NQ)
                    for idx in range(3):
                        tg = self.rot[idx % len(self.rot)]
                        if tg == "qkvps":
                            ps = psum_qkv.tile(
                                [128, NQ], FP32, tag="qkvps", name="qkvps"
                            )
                        else:
                            ps = psum_s.tile(
                                [128, NQ], FP32, tag="spsX", name="spsX",
                                bufs=1,
                            )
                        for k in range(CT):
                            nc.tensor.matmul(
                                ps,
                                self.wq[:, k, idx, :],
                                xc[:, k, :],
                                start=(k == 0),
                                stop=(k == CT - 1),
                            )
                        if idx == 0:      # q -> hi + lo fp8
                            nc.vector.tensor_copy(self.qTt[:, 0, js], ps)
                            nc.vector.tensor_tensor(
                                self.qTt[:, 1, js], ps, self.qTt[:, 0, js],
                                SUB,
                            )
                        elif idx == 1:    # k -> fp8
                            nc.vector.tensor_copy(self.kTt[:, js], ps)
                        else:             # v -> fp32r (transposed later)
                            nc.vector.tensor_copy(self.vTt[:, js], ps)

                def finish(self):
                    # v transposes; vaug = [v_hi d0-63 | ones | v_lo d1-63]
                    vaugs = []
                    for a in range(2):
                        vaug = hpool.tile([128, NT, 128], FP8, tag=f"vaug{a}")
                        nc.vector.memset(vaug[:, :, D:D + 1], 1.0)
                        vaugs.append(vaug)
                    for t0 in range(0, NT, 8):
                        pts = [
                            psum_av.tile(
                                [128, 8, D], FP32R, tag="av", name=f"pt{a}",
                                bufs=1,
                            )
                            for a in range(2)
                        ]
                        for g in range(8):
                            t = t0 + g
                            for a in range(2):
                                lo = a * D
                                nc.tensor.transpose(
                                    pts[a][:, g, :],
                                    self.vTt[lo:lo + D,
                                             t * 128:(t + 1) * 128],
                                    ident[lo:lo + D, lo:lo + D],
                                )
                        for a in range(2):
                            nc.vector.tensor_copy(
                                vaugs[a][:, t0:t0 + 8, 0:D], pts[a]
                            )
                            nc.vector.tensor_tensor(
                                vaugs[a][:, t0:t0 + 8, D + 1:128],
                                pts[a][:, :, 1:D],
                                vaugs[a][:, t0:t0 + 8, 1:D],
                                SUB,
                            )
                    return self.qTt, self.kTt, vaugs

            def emit_attention(hp, qTt, kTt, vaugs, qkv_cb=None, post_j=None):
                # ---- attention per q-chunk. aT tiles hold the full 128-row
                # [hi | denom*r | lo] block; proj contracts the doubled
                # channel dim with host-duplicated w_proj rows.
                aTts = [
                    hpool.tile([128, N], FP32R, tag=f"aTt{a}", name=f"aTt{a}")
                    for a in range(2)
                ]
                for j in range(NCH):
                    js = slice(j * NQ, (j + 1) * NQ)
                    expSs = [
                        spool.tile(
                            [128, NT, NQ], FP8,
                            tag=f"expS{a}", name=f"expS{a}", bufs=2,
                        )
                        for a in range(2)
                    ]
                    def s_mm(out_ap, a, t):
                        lo = a * D
                        kap = kTt[lo:lo + D, t * 128:(t + 1) * 128]
                        nc.tensor.matmul(
                            out_ap,
                            _dup2(kap),
                            qTt[lo:lo + D, :, js],
                            start=True,
                            stop=True,
                            perf_mode=DR,
                        )

                    def emit_act_group(g, heads=(0, 1)):
                        # tiles (2g, 2g+1) -> ScalarE exp, own sps0/1 ring
                        t = 2 * g
                        sps_ab = {
                            a: psum_s.tile(
                                [128, 2, NQ], FP32, tag=f"sps{a}",
                                name=f"sps{a}", bufs=1,
                            )
                            for a in heads
                        }
                        for u in range(2):
                            for a in heads:
                                s_mm(sps_ab[a][:, u, :], a, t + u)
                        for a in heads:
                            nc.scalar.activation(
                                out=expSs[a][:, t:t + 2, :],
                                in_=sps_ab[a][:, :, :],
                                func=EXP,
                                scale=SCALE,
                            )

                    def emit_schr_tile(t, heads=(0, 1)):
                        # kv-tile on the DVE schraudolph ring. When both
                        # heads run, their S psums share one 2-bank spsX
                        # tile so the DVE affine is a single [128,2,512] op.
                        spsx = psum_s.tile(
                            [128, 2, NQ], FP32, tag="spsX",
                            name="spsX", bufs=1,
                        )
                        for i, a in enumerate(heads):
                            s_mm(spsx[:, i, :], a, t)
                        i16 = small.tile(
                            [128, 2, NQ], I16, tag="i16", name="i16",
                        )
                        n_h = len(heads)
                        nc.vector.tensor_scalar(
                            i16[:, 0:n_h, :], spsx[:, 0:n_h, :],
                            SCHR_A, SCHR_B, MULT, ADD,
                        )
                        for i, a in enumerate(heads):
                            nc.gpsimd.tensor_copy(
                                expSs[a][:, t:t + 1, :],
                                i16[:, i, :].bitcast(BF16),
                            )

                    def emit_act_single(t, a):
                        # one kv-tile, one head on ScalarE
                        sps = psum_s.tile(
                            [128, 2, NQ], FP32, tag=f"sps{a}",
                            name=f"sps{a}", bufs=1,
                        )
                        s_mm(sps[:, 0, :], a, t)
                        nc.scalar.activation(
                            out=expSs[a][:, t:t + 1, :],
                            in_=sps[:, 0:1, :],
                            func=EXP,
                            scale=SCALE,
                        )

                    # head0: act tiles 0-9 + single 10, schr 11-15;
                    # head1: act tiles 0-11, schr 12-15. Tiles 12-15 run
                    # both heads in one paired DVE affine; the asymmetric
                    # split moves half a tile of exp off the ScalarE pacer.
                    emit_act_group(0)
                    emit_schr_tile(12)
                    emit_act_group(1)
                    emit_schr_tile(13)
                    emit_act_group(2)
                    odd = (hp + j) % 2 == 1
                    if odd:
                        emit_schr_tile(11, heads=(0,))
                    emit_act_group(3)
                    emit_schr_tile(14)
                    emit_act_group(4)
                    emit_schr_tile(15)
                    if odd:
                        emit_act_group(5, heads=(1,))
                        emit_act_single(10, 0)
                    else:
                        emit_act_group(5)
                    avs = []
                    for a in range(2):
                        av = psum_av.tile([128, NQ], FP32, tag="av", bufs=1)
                        for u in range(NT // 2):
                            nc.tensor.matmul(
                                av,
                                vaugs[a][:, 2 * u:2 * u + 2, :],
                                expSs[a][:, 2 * u:2 * u + 2, :],
                                start=(u == 0),
                                stop=(u == NT // 2 - 1),
                                perf_mode=DR,
                            )
                        avs.append(av)
                    # next pair's qkv copies go ahead of the normalize in
                    # the DVE stream: they only wait on quick qkv matmuls,
                    # while the normalize waits on the whole AV chain
                    if qkv_cb is not None:
                        qkv_cb(j)
                    for a in range(2):
                        av = avs[a]
                        # normalize all 128 rows at once (partition count is
                        # free); row 64 becomes denom*recip = 1 and hits a
                        # zero row of the extended w_proj
                        recip = small.tile([1, NQ], FP32, tag="recip")
                        nc.vector.reciprocal(recip, av[D:D + 1, :])
                        bc128 = small.tile([128, NQ], FP32, tag="bc128")
                        nc.gpsimd.partition_broadcast(bc128, recip)
                        nc.vector.tensor_tensor(
                            aTts[a][:, js], av, bc128, MULT
                        )
                    # half-pair aT stores; the last pair skips them (its
                    # aT feeds the proj straight from SBUF)
                    if j % 2 == 1 and hp < H // 2 - 1:
                        js2 = slice((j - 1) * NQ, (j + 1) * NQ)
                        for a in range(2):
                            h = 2 * hp + a
                            nc.sync.dma_start(
                                out=aT_d[h * 128:(h + 1) * 128, js2],
                                in_=aTts[a][:, js2],
                            )
                    if post_j is not None:
                        post_j(j)
                return aTts

            # proj weights/bias DMA'd up front on the gpsimd DMA queue so
            # they don't delay the first pair's x/wq loads on SP
            bias_bc = small.tile([128, C], FP32, tag="bias", bufs=1)
            nc.gpsimd.dma_start(out=bias_bc, in_=_bc_ap(b_proj[:], 128))
            w_projT = small.tile([128, 2 * CT, C], FP32R, tag="wproj",
                                 bufs=1)
            nc.gpsimd.dma_start(
                out=w_projT, in_=wpr.rearrange("(ko p) o -> p ko o", p=128)
            )

            NO = 384

            def emit_proj(i2_list, psum_tags, aT5=None):
                for n_p, i2 in enumerate(i2_list):
                    a_sb = small.tile(
                        [128, 2 * CT, 256], FP32R, tag="a_sb", bufs=2
                    )
                    nc.sync.dma_start(
                        out=a_sb,
                        in_=aT_d[:, i2 * 256:(i2 + 1) * 256].rearrange(
                            "(ko p) n -> p ko n", p=128
                        ),
                    )
                    for ii in range(2):
                        i = 2 * i2 + ii
                        y_sb = small.tile([128, C], FP32, tag="y_sb", bufs=2)
                        for half in range(2):
                            tg = psum_tags[(n_p * 4 + ii * 2 + half)
                                           % len(psum_tags)]
                            if tg == "qkvps":
                                psf = psum_qkv.tile(
                                    [128, NQ], FP32, tag="qkvps",
                                    name="qkvps")
                            elif tg == "spsX":
                                psf = psum_s.tile(
                                    [128, NQ], FP32, tag="spsX",
                                    name="spsX", bufs=1)
                            elif tg in ("sps0", "sps1"):
                                psf = psum_s.tile(
                                    [128, 2, NQ], FP32, tag=tg,
                                    name=tg, bufs=1)[:, 0, :]
                            else:
                                psf = psum_av.tile(
                                    [128, NQ], FP32, tag="av", bufs=1)
                            ps = psf[:, 0:NO]
                            isl = slice(i2 * 256 + ii * 128,
                                        i2 * 256 + (ii + 1) * 128)
                            for k in range(2 * CT):
                                # last pair's chunks come straight from its
                                # SBUF aT tiles - no DRAM roundtrip wait
                                lhs = (a_sb[:, k, ii * 128:(ii + 1) * 128]
                                       if k < 10 else aT5[k - 10][:, isl])
                                nc.tensor.matmul(
                                    ps,
                                    lhs,
                                    w_projT[:, k, half * NO:(half + 1) * NO],
                                    start=(k == 0),
                                    stop=(k == 2 * CT - 1),
                                )
                            nc.vector.tensor_add(
                                y_sb[:, half * NO:(half + 1) * NO], ps,
                                bias_bc[:, half * NO:(half + 1) * NO]
                            )
                        nc.sync.dma_start(
                            out=y[i * 128:(i + 1) * 128, :],
                            in_=y_sb,
                        )

            def last_pair_post_j(j):
                # overlap most of the projection with the last pair's
                # attention, using the qkv psum bank plus the freed sps1 ring
                pass

            cur = QkvPhase(0, rot=("qkvps", "spsX"))
            for j in range(NCH):
                cur.emit_j(j)
            tiles = cur.finish()
            aT5 = None
            for hp in range(H // 2):
                nxt = QkvPhase(hp + 1) if hp + 1 < H // 2 else None
                ret = emit_attention(
                    hp, *tiles,
                    qkv_cb=nxt.emit_j if nxt is not None else None,
                    post_j=last_pair_post_j if hp == H // 2 - 1 else None,
                )
                if hp == H // 2 - 1:
                    aT5 = ret
                if nxt is not None:
                    tiles = nxt.finish()

            # ---------- rest of the projection; attention psums now free
            emit_proj([0, 1, 2, 3, 4, 5, 6, 7],
                      ["qkvps", "sps0", "sps1", "spsX", "av"], aT5=aT5)
        const_cm.__exit__(None, None, None)
    lp.__exit__(None, None, None)

    nc.finalize()
    return nc


def get_nc():
    global _CACHED_NC
    if _CACHED_NC is None:
        _CACHED_NC = build()
    return _CACHED_NC


LAST_RESULT = None


def kernel(x, w_qkv, w_proj, b_proj, **run_kwargs):
    x = np.ascontiguousarray(np.asarray(x, dtype=np.float32))
    w_qkv = np.ascontiguousarray(np.asarray(w_qkv, dtype=np.float32))
    w_proj = np.ascontiguousarray(np.asarray(w_proj, dtype=np.float32))
    b_proj = np.ascontiguousarray(np.asarray(b_proj, dtype=np.float32))
    assert x.shape == (B, N, C)

    nc = get_nc()
    w_qkvT = np.ascontiguousarray(w_qkv.T)
    # extended proj weights: per head the aT block is
    # [hi d0-63 | denom*r (==1) | v_lo d1-63]; duplicate w rows for the lo
    # part, zero the denom row.
    w_ext = np.zeros((2 * C, C), dtype=np.float32)
    for h in range(H):
        blk = w_proj[:, h * D:(h + 1) * D]          # [C_out, 64]
        w_ext[h * 128:h * 128 + 64, :] = blk.T
        w_ext[h * 128 + 65:h * 128 + 128, :] = blk.T[1:64]
    w_projT = np.ascontiguousarray(w_ext)
    b_eff = np.ascontiguousarray(b_proj.astype(np.float32))
    in_maps = [
        {
            "xT": np.ascontiguousarray(x[i].T),
            "w_qkvT": w_qkvT,
            "w_projT": w_projT,
            "b_proj": b_eff,
        }
        for i in range(B)
    ]
    res = run_bass_kernel_spmd(nc, in_maps, list(range(B)), **run_kwargs)
    global LAST_RESULT
    LAST_RESULT = res
    out = np.stack([res.results[i]["y"] for i in range(B)], axis=0)
    return out


if __name__ == "__main__":
    rng = np.random.default_rng(0)
    x = rng.standard_normal((B, N, C), dtype=np.float32)
    w_qkv = (rng.standard_normal((F, C)) * 0.02).astype(np.float32)
    w_proj = (rng.standard_normal((C, C)) * 0.02).astype(np.float32)
    b_proj = (rng.standard_normal((C,)) * 0.02).astype(np.float32)
    out = kernel(x=x, w_qkv=w_qkv, w_proj=w_proj, b_proj=b_proj)
    print("out", out.shape, out.dtype, float(np.abs(out).max()))


# revision 36
# speedup vs baseline: 1.0050x; 1.0012x over previous
"""Multi-head attention Trainium2 Bass kernel (fp8-DoubleRow version).

Problem: B=8, N=2048, C=768, H=12 heads, D=64 head dim.
  qkv = x @ w_qkv.T          -> [B, N, 3C]
  per head: softmax(q k^T / sqrt(D)) @ v
  y = attn_out @ w_proj.T + b_proj

Sharding: data parallel over batch - one batch element per NeuronCore.

Mixed-precision strategy (cost model: matmul = out_free x pe_cycle x cpr,
fp8 DoubleRow cpr=0.5 vs fp32r 1.0):
  - QKV: fp32r (exact q/k/v; fp8 here costs too much accuracy).
  - S = k^T q: fp8 DoubleRow. lhsT = k8 duplicated via a stride-0 block dim,
    rhs blocks = (q_hi, q_lo) -> S = k8^T (q_hi + q_lo): q at ~14-bit
    precision, k at fp8, half the fp32r PE cost.
  - exp: ~11.75/16 kv-tiles on ScalarE (activation Exp -> fp8 out), the
    rest via a Schraudolph fast-exp: DVE affine to int16 (bf16 exponent
    bits), gpsimd converts bf16 -> fp8e4m3. Both heads of a schr tile
    share one 2-bank psum so the affine is a single [128,2,512] op; one
    extra tile alternates engines per chunk to balance ScalarE/DVE.
  - AV: fp8 DoubleRow pairing kv-tiles. lhsT M-dim = [v_hi d0-63 | ones |
    v_lo d1-63]: v at ~14-bit, denominator row free (psum row 64).
  - normalize: one reciprocal of the denominator row + partition_broadcast
    + one full-128-row multiply (partition count is free in the cost
    model). The hi+lo recombination is deferred to the projection, which
    contracts 1536 channels against host-duplicated w_proj rows; the
    denominator row (denom*recip == 1) hits a zeroed weight row.
  - proj: fp32r over the extended 1536-channel aT, tail-only with a wide
    psum rotation over the freed attention banks.

Scheduling: engines execute their streams in order, so emission order is
the schedule. QKV chunks of pair hp+1 are emitted after each q-chunk of
pair hp's attention (j-interleave); the Schraudolph groups sit on their
own psum ring so the DVE never gates the ScalarE act ring.
"""

import numpy as np

import concourse.bass as bass
import concourse.mybir as mybir
import concourse.tile as tile
from concourse import bacc
from concourse.bass_utils import run_bass_kernel_spmd
from concourse.masks import make_identity

B, N, C, H = 8, 2048, 768, 12
D = C // H            # 64
F = 3 * C             # 2304
NT = N // 128         # 16 seq tiles
CT = C // 128         # 6 channel tiles
NQ = 512              # query-chunk width (1 psum bank of fp32)
NCH = N // NQ         # 4 chunks
SCALE = float(D) ** -0.5

FP32 = mybir.dt.float32
FP32R = mybir.dt.float32r
FP8 = mybir.dt.float8e4
I16 = mybir.dt.int16
BF16 = mybir.dt.bfloat16
EXP = mybir.ActivationFunctionType.Exp
DR = mybir.MatmulPerfMode.DoubleRow
ADD = mybir.AluOpType.add
SUB = mybir.AluOpType.subtract
MULT = mybir.AluOpType.mult

# Schraudolph constants: bf16 bits = 128*log2(exp(s_eff)) + 127*128,
# s_eff = S_psum * SCALE -> bits = S * (128*SCALE/ln2) + 16256.
SCHR_A = 128.0 * SCALE / float(np.log(2.0))
SCHR_B = 16256.0
N_SCHR_GROUPS = 2      # of 8 groups of 2 kv-tiles: last N on DVE+Pool

_CACHED_NC = None


def _dup2(ap):
    """Insert a stride-0 [0,2] block dim after the partition dim."""
    return bass.AP(
        tensor=ap.tensor,
        offset=ap.offset,
        ap=[list(ap.ap[0]), [0, 2]] + [list(d) for d in ap.ap[1:]],
    )


def _bc_ap(dram_ap, parts):
    """Partition-broadcast a 1-D DRAM AP to [parts, len] via stride-0."""
    return bass.AP(
        tensor=dram_ap.tensor,
        offset=dram_ap.offset,
        ap=[[0, parts]] + [list(p) for p in dram_ap.ap],
    )


def build():
    nc = bacc.Bacc()
    x = nc.dram_tensor("xT", [C, N], FP32, kind="ExternalInput")
    w_qkv = nc.dram_tensor("w_qkvT", [C, F], FP32, kind="ExternalInput")
    w_proj = nc.dram_tensor("w_projT", [2 * C, C], FP32, kind="ExternalInput")
    b_proj = nc.dram_tensor("b_proj", [C], FP32, kind="ExternalInput")
    y = nc.dram_tensor("y", [N, C], FP32, kind="ExternalOutput")
    C2 = 2 * C
    aT_d = nc.dram_tensor("aT_scratch", [C2, N], FP32R)

    xr = x[:, :].bitcast(FP32R)
    wqr = w_qkv[:, :].bitcast(FP32R)
    wpr = w_proj[:, :].bitcast(FP32R)

    lp = nc.allow_low_precision("fp8 attention with hi/lo compensation")
    lp.__enter__()
    with tile.TileContext(nc) as tc:
        const_cm = tc.tile_pool(name="const", bufs=1)
        const = const_cm.__enter__()
        ident_f = const.tile([128, 128], FP32)
        make_identity(nc, ident_f)
        ident = const.tile([128, 128], FP32R)
        nc.vector.tensor_copy(ident, ident_f)
        xr3 = xr.rearrange("(ko p) n -> p ko n", p=128)
        wqr3 = wqr.rearrange("(ko p) f -> p ko f", p=128)

        with tc.tile_pool(name="hpool", bufs=2) as hpool, \
             tc.tile_pool(name="spool", bufs=1) as spool, \
             tc.tile_pool(name="small", bufs=2) as small, \
             tc.tile_pool(name="psum_s", bufs=2, space="PSUM") as psum_s, \
             tc.tile_pool(name="psum_av", bufs=2, space="PSUM") as psum_av, \
             tc.tile_pool(name="psum_qkv", bufs=1, space="PSUM") as psum_qkv:

            class QkvPhase:
                """QKV chunks for one head pair, emitted j-at-a-time so the
                matmuls/copies interleave with the previous pair's
                attention on every engine stream."""

                def __init__(self, hp, rot=("qkvps",)):
                    self.hp = hp
                    self.rot = rot
                    self.xc0 = None
                    if hp == 0:
                        # prefetch the first x chunk ahead of the wq loads
                        # so the first qkv matmul chain starts ~5us sooner
                        self.xc0 = hpool.tile(
                            [128, CT, NQ], FP32R, tag="xc", name="xc",
                        )
                        nc.sync.dma_start(out=self.xc0, in_=xr3[:, :, 0:NQ])
                    self.wq = hpool.tile(
                        [128, CT, 3, 128], FP32R, tag="wq", name="wq", bufs=1,
                    )
                    for idx, m in enumerate((hp, CT + hp, 2 * CT + hp)):
                        nc.sync.dma_start(
                            out=self.wq[:, :, idx, :],
                            in_=wqr3[:, :, m * 128:(m + 1) * 128],
                        )
                    self.qTt = hpool.tile([128, 2, N], FP8, tag="qT")
                    self.kTt = hpool.tile([128, N], FP8, tag="kT")
                    self.vTt = hpool.tile([128, N], FP32R, tag="vT", bufs=1)

                def emit_j(self, j):
                    if j == 0 and self.xc0 is not None:
                        xc = self.xc0
                    else:
                        xc = hpool.tile(
                            [128, CT, NQ], FP32R, tag="xc", name="xc",
                        )
                        nc.sync.dma_start(
                            out=xc, in_=xr3[:, :, j * NQ:(j + 1) * NQ]
                        )
                    js = slice(j * NQ, (j + 1) * NQ)
                    for idx in range(3):
                        tg = self.rot[idx % len(self.rot)]
                        if tg == "qkvps":
                            ps = psum_qkv.tile(
                                [128, NQ], FP32, tag="qkvps", name="qkvps"
                            )
                        else:
                            ps = psum_s.tile(
                                [128, NQ], FP32, tag="spsX", name="spsX",
                                bufs=1,
                            )
                        for k in range(CT):
                            nc.tensor.matmul(
                                ps,
                                self.wq[:, k, idx, :],
                                xc[:, k, :],
                                start=(k == 0),
                                stop=(k == CT - 1),
                            )
                        if idx == 0:      # q -> hi + lo fp8
                            nc.vector.tensor_copy(self.qTt[:, 0, js], ps)
                            nc.vector.tensor_tensor(
                                self.qTt[:, 1, js], ps, self.qTt[:, 0, js],
                                SUB,
                            )
                        elif idx == 1:    # k -> fp8
                            nc.vector.tensor_copy(self.kTt[:, js], ps)
                        else:             # v -> fp32r (transposed later)
                            nc.vector.tensor_copy(self.vTt[:, js], ps)

                def finish(self):
                    # v transposes; vaug = [v_hi d0-63 | ones | v_lo d1-63]
                    vaugs = []
                    for a in range(2):
                        vaug = hpool.tile([128, NT, 128], FP8, tag=f"vaug{a}")
                        nc.vector.memset(vaug[:, :, D:D + 1], 1.0)
                        vaugs.append(vaug)
                    for t0 in range(0, NT, 8):
                        pts = [
                            psum_av.tile(
                                [128, 8, D], FP32R, tag="av", name=f"pt{a}",
                                bufs=1,
                            )
                            for a in range(2)
                        ]
                        for g in range(8):
                            t = t0 + g
                            for a in range(2):
                                lo = a * D
                                nc.tensor.transpose(
                                    pts[a][:, g, :],
                                    self.vTt[lo:lo + D,
                                             t * 128:(t + 1) * 128],
                                    ident[lo:lo + D, lo:lo + D],
                                )
                        for a in range(2):
                            nc.vector.tensor_copy(
                                vaugs[a][:, t0:t0 + 8, 0:D], pts[a]
                            )
                            nc.vector.tensor_tensor(
                                vaugs[a][:, t0:t0 + 8, D + 1:128],
                                pts[a][:, :, 1:D],
                                vaugs[a][:, t0:t0 + 8, 1:D],
                                SUB,
                            )
                    return self.qTt, self.kTt, vaugs

            def emit_attention(hp, qTt, kTt, vaugs, qkv_cb=None, post_j=None):
                # ---- attention per q-chunk. aT tiles hold the full 128-row
                # [hi | denom*r | lo] block; proj contracts the doubled
                # channel dim with host-duplicated w_proj rows.
                aTts = [
                    hpool.tile([128, N], FP32R, tag=f"aTt{a}", name=f"aTt{a}")
                    for a in range(2)
                ]
                for j in range(NCH):
                    js = slice(j * NQ, (j + 1) * NQ)
                    expSs = [
                        spool.tile(
                            [128, NT, NQ], FP8,
                            tag=f"expS{a}", name=f"expS{a}", bufs=2,
                        )
                        for a in range(2)
                    ]
                    def s_mm(out_ap, a, t):
                        lo = a * D
                        kap = kTt[lo:lo + D, t * 128:(t + 1) * 128]
                        nc.tensor.matmul(
                            out_ap,
                            _dup2(kap),
                            qTt[lo:lo + D, :, js],
                            start=True,
                            stop=True,
                            perf_mode=DR,
                        )

                    def emit_act_group(g, heads=(0, 1)):
                        # tiles (2g, 2g+1) -> ScalarE exp, own sps0/1 ring
                        t = 2 * g
                        sps_ab = {
                            a: psum_s.tile(
                                [128, 2, NQ], FP32, tag=f"sps{a}",
                                name=f"sps{a}", bufs=1,
                            )
                            for a in heads
                        }
                        for u in range(2):
                            for a in reversed(heads):
                                s_mm(sps_ab[a][:, u, :], a, t + u)
                        for a in reversed(heads):
                            nc.scalar.activation(
                                out=expSs[a][:, t:t + 2, :],
                                in_=sps_ab[a][:, :, :],
                                func=EXP,
                                scale=SCALE,
                            )

                    def emit_schr_tile(t, heads=(0, 1)):
                        # kv-tile on the DVE schraudolph ring. When both
                        # heads run, their S psums share one 2-bank spsX
                        # tile so the DVE affine is a single [128,2,512] op.
                        spsx = psum_s.tile(
                            [128, 2, NQ], FP32, tag="spsX",
                            name="spsX", bufs=1,
                        )
                        for i, a in enumerate(heads):
                            s_mm(spsx[:, i, :], a, t)
                        i16 = small.tile(
                            [128, 2, NQ], I16, tag="i16", name="i16",
                        )
                        n_h = len(heads)
                        nc.vector.tensor_scalar(
                            i16[:, 0:n_h, :], spsx[:, 0:n_h, :],
                            SCHR_A, SCHR_B, MULT, ADD,
                        )
                        for i, a in enumerate(heads):
                            nc.gpsimd.tensor_copy(
                                expSs[a][:, t:t + 1, :],
                                i16[:, i, :].bitcast(BF16),
                            )

                    def emit_act_single(t, a):
                        # one kv-tile, one head on ScalarE
                        sps = psum_s.tile(
                            [128, 2, NQ], FP32, tag=f"sps{a}",
                            name=f"sps{a}", bufs=1,
                        )
                        s_mm(sps[:, 0, :], a, t)
                        nc.scalar.activation(
                            out=expSs[a][:, t:t + 1, :],
                            in_=sps[:, 0:1, :],
                            func=EXP,
                            scale=SCALE,
                        )

                    # head0: act tiles 0-9 + single 10, schr 11-15;
                    # head1: act tiles 0-11, schr 12-15. Tiles 12-15 run
                    # both heads in one paired DVE affine; the asymmetric
                    # split moves half a tile of exp off the ScalarE pacer.
                    emit_act_group(0)
                    emit_schr_tile(12)
                    emit_act_group(1)
                    emit_schr_tile(13)
                    emit_act_group(2)
                    odd = (hp + j) % 2 == 1
                    if odd:
                        emit_schr_tile(11, heads=(0,))
                    emit_act_group(3)
                    emit_schr_tile(14)
                    emit_act_group(4)
                    emit_schr_tile(15)
                    if odd:
                        emit_act_group(5, heads=(1,))
                        emit_act_single(10, 0)
                    else:
                        emit_act_group(5)
                    avs = []
                    for a in range(2):
                        av = psum_av.tile([128, NQ], FP32, tag="av", bufs=1)
                        for u in range(NT // 2):
                            nc.tensor.matmul(
                                av,
                                vaugs[a][:, 2 * u:2 * u + 2, :],
                                expSs[a][:, 2 * u:2 * u + 2, :],
                                start=(u == 0),
                                stop=(u == NT // 2 - 1),
                                perf_mode=DR,
                            )
                        avs.append(av)
                    # next pair's qkv copies go ahead of the normalize in
                    # the DVE stream: they only wait on quick qkv matmuls,
                    # while the normalize waits on the whole AV chain
                    if qkv_cb is not None:
                        qkv_cb(j)
                    for a in range(2):
                        av = avs[a]
                        # normalize all 128 rows at once (partition count is
                        # free); row 64 becomes denom*recip = 1 and hits a
                        # zero row of the extended w_proj
                        recip = small.tile([1, NQ], FP32, tag="recip")
                        nc.vector.reciprocal(recip, av[D:D + 1, :])
                        bc128 = small.tile([128, NQ], FP32, tag="bc128")
                        nc.gpsimd.partition_broadcast(bc128, recip)
                        nc.vector.tensor_tensor(
                            aTts[a][:, js], av, bc128, MULT
                        )
                    # half-pair aT stores; the last pair skips them (its
                    # aT feeds the proj straight from SBUF)
                    if j % 2 == 1 and hp < H // 2 - 1:
                        js2 = slice((j - 1) * NQ, (j + 1) * NQ)
                        for a in range(2):
                            h = 2 * hp + a
                            nc.sync.dma_start(
                                out=aT_d[h * 128:(h + 1) * 128, js2],
                                in_=aTts[a][:, js2],
                            )
                    if post_j is not None:
                        post_j(j)
                return aTts

            # proj weights/bias DMA'd up front on the gpsimd DMA queue so
            # they don't delay the first pair's x/wq loads on SP
            bias_bc = small.tile([128, C], FP32, tag="bias", bufs=1)
            nc.gpsimd.dma_start(out=bias_bc, in_=_bc_ap(b_proj[:], 128))
            w_projT = small.tile([128, 2 * CT, C], FP32R, tag="wproj",
                                 bufs=1)
            nc.gpsimd.dma_start(
                out=w_projT, in_=wpr.rearrange("(ko p) o -> p ko o", p=128)
            )

            NO = 384

            def emit_proj(i2_list, psum_tags, aT5=None):
                for n_p, i2 in enumerate(i2_list):
                    a_sb = small.tile(
                        [128, 2 * CT, 256], FP32R, tag="a_sb", bufs=2
                    )
                    nc.sync.dma_start(
                        out=a_sb,
                        in_=aT_d[:, i2 * 256:(i2 + 1) * 256].rearrange(
                            "(ko p) n -> p ko n", p=128
                        ),
                    )
                    for ii in range(2):
                        i = 2 * i2 + ii
                        y_sb = small.tile([128, C], FP32, tag="y_sb", bufs=2)
                        for half in range(2):
                            tg = psum_tags[(n_p * 4 + ii * 2 + half)
                                           % len(psum_tags)]
                            if tg == "qkvps":
                                psf = psum_qkv.tile(
                                    [128, NQ], FP32, tag="qkvps",
                                    name="qkvps")
                            elif tg == "spsX":
                                psf = psum_s.tile(
                                    [128, NQ], FP32, tag="spsX",
                                    name="spsX", bufs=1)
                            elif tg in ("sps0", "sps1"):
                                psf = psum_s.tile(
                                    [128, 2, NQ], FP32, tag=tg,
                                    name=tg, bufs=1)[:, 0, :]
                            else:
                                psf = psum_av.tile(
                                    [128, NQ], FP32, tag="av", bufs=1)
                            ps = psf[:, 0:NO]
                            isl = slice(i2 * 256 + ii * 128,
                                        i2 * 256 + (ii + 1) * 128)
                            for k in range(2 * CT):
                                # last pair's chunks come straight from its
                                # SBUF aT tiles - no DRAM roundtrip wait
                                lhs = (a_sb[:, k, ii * 128:(ii + 1) * 128]
                                       if k < 10 else aT5[k - 10][:, isl])
                                nc.tensor.matmul(
                                    ps,
                                    lhs,
                                    w_projT[:, k, half * NO:(half + 1) * NO],
                                    start=(k == 0),
                                    stop=(k == 2 * CT - 1),
                                )
                            nc.vector.tensor_add(
                                y_sb[:, half * NO:(half + 1) * NO], ps,
                                bias_bc[:, half * NO:(half + 1) * NO]
                            )
                        nc.sync.dma_start(
                            out=y[i * 128:(i + 1) * 128, :],
                            in_=y_sb,
                        )

            def last_pair_post_j(j):
                # overlap most of the projection with the last pair's
                # attention, using the qkv psum bank plus the freed sps1 ring
                pass

            cur = QkvPhase(0, rot=("qkvps", "spsX"))
            for j in range(NCH):
                cur.emit_j(j)
            tiles = cur.finish()
            aT5 = None
            for hp in range(H // 2):
                nxt = QkvPhase(hp + 1) if hp + 1 < H // 2 else None
                ret = emit_attention(
                    hp, *tiles,
                    qkv_cb=nxt.emit_j if nxt is not None else None,
                    post_j=last_pair_post_j if hp == H // 2 - 1 else None,
                )
                if hp == H // 2 - 1:
                    aT5 = ret
                if nxt is not None:
                    tiles = nxt.finish()

            # ---------- rest of the projection; attention psums now free
            emit_proj([0, 1, 2, 3, 4, 5, 6, 7],
                      ["qkvps", "sps0", "sps1", "spsX", "av"], aT5=aT5)
        const_cm.__exit__(None, None, None)
    lp.__exit__(None, None, None)

    nc.finalize()
    return nc


def get_nc():
    global _CACHED_NC
    if _CACHED_NC is None:
        _CACHED_NC = build()
    return _CACHED_NC


LAST_RESULT = None


def kernel(x, w_qkv, w_proj, b_proj, **run_kwargs):
    x = np.ascontiguousarray(np.asarray(x, dtype=np.float32))
    w_qkv = np.ascontiguousarray(np.asarray(w_qkv, dtype=np.float32))
    w_proj = np.ascontiguousarray(np.asarray(w_proj, dtype=np.float32))
    b_proj = np.ascontiguousarray(np.asarray(b_proj, dtype=np.float32))
    assert x.shape == (B, N, C)

    nc = get_nc()
    w_qkvT = np.ascontiguousarray(w_qkv.T)
    # extended proj weights: per head the aT block is
    # [hi d0-63 | denom*r (==1) | v_lo d1-63]; duplicate w rows for the lo
    # part, zero the denom row.
    w_ext = np.zeros((2 * C, C), dtype=np.float32)
    for h in range(H):
        blk = w_proj[:, h * D:(h + 1) * D]          # [C_out, 64]
        w_ext[h * 128:h * 128 + 64, :] = blk.T
        w_ext[h * 128 + 65:h * 128 + 128, :] = blk.T[1:64]
    w_projT = np.ascontiguousarray(w_ext)
    b_eff = np.ascontiguousarray(b_proj.astype(np.float32))
    in_maps = [
        {
            "xT": np.ascontiguousarray(x[i].T),
            "w_qkvT": w_qkvT,
            "w_projT": w_projT,
            "b_proj": b_eff,
        }
        for i in range(B)
    ]
    res = run_bass_kernel_spmd(nc, in_maps, list(range(B)), **run_kwargs)
    global LAST_RESULT
    LAST_RESULT = res
    out = np.stack([res.results[i]["y"] for i in range(B)], axis=0)
    return out


if __name__ == "__main__":
    rng = np.random.default_rng(0)
    x = rng.standard_normal((B, N, C), dtype=np.float32)
    w_qkv = (rng.standard_normal((F, C)) * 0.02).astype(np.float32)
    w_proj = (rng.standard_normal((C, C)) * 0.02).astype(np.float32)
    b_proj = (rng.standard_normal((C,)) * 0.02).astype(np.float32)
    out = kernel(x=x, w_qkv=w_qkv, w_proj=w_proj, b_proj=b_proj)
    print("out", out.shape, out.dtype, float(np.abs(out).max()))


# revision 37
# speedup vs baseline: 1.0058x; 1.0007x over previous
"""Multi-head attention Trainium2 Bass kernel (fp8-DoubleRow version).

Problem: B=8, N=2048, C=768, H=12 heads, D=64 head dim.
  qkv = x @ w_qkv.T          -> [B, N, 3C]
  per head: softmax(q k^T / sqrt(D)) @ v
  y = attn_out @ w_proj.T + b_proj

Sharding: data parallel over batch - one batch element per NeuronCore.

Mixed-precision strategy (cost model: matmul = out_free x pe_cycle x cpr,
fp8 DoubleRow cpr=0.5 vs fp32r 1.0):
  - QKV: fp32r (exact q/k/v; fp8 here costs too much accuracy).
  - S = k^T q: fp8 DoubleRow. lhsT = k8 duplicated via a stride-0 block dim,
    rhs blocks = (q_hi, q_lo) -> S = k8^T (q_hi + q_lo): q at ~14-bit
    precision, k at fp8, half the fp32r PE cost.
  - exp: ~11.75/16 kv-tiles on ScalarE (activation Exp -> fp8 out), the
    rest via a Schraudolph fast-exp: DVE affine to int16 (bf16 exponent
    bits), gpsimd converts bf16 -> fp8e4m3. Both heads of a schr tile
    share one 2-bank psum so the affine is a single [128,2,512] op; one
    extra tile alternates engines per chunk to balance ScalarE/DVE.
  - AV: fp8 DoubleRow pairing kv-tiles. lhsT M-dim = [v_hi d0-63 | ones |
    v_lo d1-63]: v at ~14-bit, denominator row free (psum row 64).
  - normalize: one reciprocal of the denominator row + partition_broadcast
    + one full-128-row multiply (partition count is free in the cost
    model). The hi+lo recombination is deferred to the projection, which
    contracts 1536 channels against host-duplicated w_proj rows; the
    denominator row (denom*recip == 1) hits a zeroed weight row.
  - proj: fp32r over the extended 1536-channel aT, tail-only with a wide
    psum rotation over the freed attention banks.

Scheduling: engines execute their streams in order, so emission order is
the schedule. QKV chunks of pair hp+1 are emitted after each q-chunk of
pair hp's attention (j-interleave); the Schraudolph groups sit on their
own psum ring so the DVE never gates the ScalarE act ring.
"""

import numpy as np

import concourse.bass as bass
import concourse.mybir as mybir
import concourse.tile as tile
from concourse import bacc
from concourse.bass_utils import run_bass_kernel_spmd
from concourse.masks import make_identity

B, N, C, H = 8, 2048, 768, 12
D = C // H            # 64
F = 3 * C             # 2304
NT = N // 128         # 16 seq tiles
CT = C // 128         # 6 channel tiles
NQ = 512              # query-chunk width (1 psum bank of fp32)
NCH = N // NQ         # 4 chunks
SCALE = float(D) ** -0.5

FP32 = mybir.dt.float32
FP32R = mybir.dt.float32r
FP8 = mybir.dt.float8e4
I16 = mybir.dt.int16
BF16 = mybir.dt.bfloat16
EXP = mybir.ActivationFunctionType.Exp
DR = mybir.MatmulPerfMode.DoubleRow
ADD = mybir.AluOpType.add
SUB = mybir.AluOpType.subtract
MULT = mybir.AluOpType.mult

# Schraudolph constants: bf16 bits = 128*log2(exp(s_eff)) + 127*128,
# s_eff = S_psum * SCALE -> bits = S * (128*SCALE/ln2) + 16256.
SCHR_A = 128.0 * SCALE / float(np.log(2.0))
SCHR_B = 16256.0
N_SCHR_GROUPS = 2      # of 8 groups of 2 kv-tiles: last N on DVE+Pool

_CACHED_NC = None


def _dup2(ap):
    """Insert a stride-0 [0,2] block dim after the partition dim."""
    return bass.AP(
        tensor=ap.tensor,
        offset=ap.offset,
        ap=[list(ap.ap[0]), [0, 2]] + [list(d) for d in ap.ap[1:]],
    )


def _bc_ap(dram_ap, parts):
    """Partition-broadcast a 1-D DRAM AP to [parts, len] via stride-0."""
    return bass.AP(
        tensor=dram_ap.tensor,
        offset=dram_ap.offset,
        ap=[[0, parts]] + [list(p) for p in dram_ap.ap],
    )


def build():
    nc = bacc.Bacc()
    x = nc.dram_tensor("xT", [C, N], FP32, kind="ExternalInput")
    w_qkv = nc.dram_tensor("w_qkvT", [C, F], FP32, kind="ExternalInput")
    w_proj = nc.dram_tensor("w_projT", [2 * C, C], FP32, kind="ExternalInput")
    b_proj = nc.dram_tensor("b_proj", [C], FP32, kind="ExternalInput")
    y = nc.dram_tensor("y", [N, C], FP32, kind="ExternalOutput")
    C2 = 2 * C
    aT_d = nc.dram_tensor("aT_scratch", [C2, N], FP32R)

    xr = x[:, :].bitcast(FP32R)
    wqr = w_qkv[:, :].bitcast(FP32R)
    wpr = w_proj[:, :].bitcast(FP32R)

    lp = nc.allow_low_precision("fp8 attention with hi/lo compensation")
    lp.__enter__()
    with tile.TileContext(nc) as tc:
        const_cm = tc.tile_pool(name="const", bufs=1)
        const = const_cm.__enter__()
        ident_f = const.tile([128, 128], FP32)
        make_identity(nc, ident_f)
        ident = const.tile([128, 128], FP32R)
        nc.vector.tensor_copy(ident, ident_f)
        xr3 = xr.rearrange("(ko p) n -> p ko n", p=128)
        wqr3 = wqr.rearrange("(ko p) f -> p ko f", p=128)

        with tc.tile_pool(name="hpool", bufs=2) as hpool, \
             tc.tile_pool(name="spool", bufs=1) as spool, \
             tc.tile_pool(name="small", bufs=2) as small, \
             tc.tile_pool(name="psum_s", bufs=2, space="PSUM") as psum_s, \
             tc.tile_pool(name="psum_av", bufs=2, space="PSUM") as psum_av, \
             tc.tile_pool(name="psum_qkv", bufs=1, space="PSUM") as psum_qkv:

            class QkvPhase:
                """QKV chunks for one head pair, emitted j-at-a-time so the
                matmuls/copies interleave with the previous pair's
                attention on every engine stream."""

                def __init__(self, hp, rot=("qkvps",)):
                    self.hp = hp
                    self.rot = rot
                    self.xc0 = None
                    if hp == 0:
                        # prefetch the first x chunk ahead of the wq loads
                        # so the first qkv matmul chain starts ~5us sooner
                        self.xc0 = hpool.tile(
                            [128, CT, NQ], FP32R, tag="xc", name="xc",
                        )
                        nc.sync.dma_start(out=self.xc0, in_=xr3[:, :, 0:NQ])
                    self.wq = hpool.tile(
                        [128, CT, 3, 128], FP32R, tag="wq", name="wq", bufs=1,
                    )
                    for idx, m in enumerate((hp, CT + hp, 2 * CT + hp)):
                        nc.sync.dma_start(
                            out=self.wq[:, :, idx, :],
                            in_=wqr3[:, :, m * 128:(m + 1) * 128],
                        )
                    self.qTt = hpool.tile([128, 2, N], FP8, tag="qT")
                    self.kTt = hpool.tile([128, N], FP8, tag="kT")
                    self.vTt = hpool.tile([128, N], FP32R, tag="vT", bufs=1)

                def emit_j(self, j):
                    if j == 0 and self.xc0 is not None:
                        xc = self.xc0
                    else:
                        xc = hpool.tile(
                            [128, CT, NQ], FP32R, tag="xc", name="xc",
                        )
                        nc.sync.dma_start(
                            out=xc, in_=xr3[:, :, j * NQ:(j + 1) * NQ]
                        )
                    js = slice(j * NQ, (j + 1) * NQ)
                    for idx in range(3):
                        tg = self.rot[idx % len(self.rot)]
                        if tg == "qkvps":
                            ps = psum_qkv.tile(
                                [128, NQ], FP32, tag="qkvps", name="qkvps"
                            )
                        else:
                            ps = psum_s.tile(
                                [128, NQ], FP32, tag="spsX", name="spsX",
                                bufs=1,
                            )
                        for k in range(CT):
                            nc.tensor.matmul(
                                ps,
                                self.wq[:, k, idx, :],
                                xc[:, k, :],
                                start=(k == 0),
                                stop=(k == CT - 1),
                            )
                        if idx == 0:      # q -> hi + lo fp8
                            nc.vector.tensor_copy(self.qTt[:, 0, js], ps)
                            nc.vector.tensor_tensor(
                                self.qTt[:, 1, js], ps, self.qTt[:, 0, js],
                                SUB,
                            )
                        elif idx == 1:    # k -> fp8
                            nc.vector.tensor_copy(self.kTt[:, js], ps)
                        else:             # v -> fp32r (transposed later)
                            nc.vector.tensor_copy(self.vTt[:, js], ps)

                def finish(self):
                    # v transposes; vaug = [v_hi d0-63 | ones | v_lo d1-63]
                    vaugs = []
                    for a in range(2):
                        vaug = hpool.tile([128, NT, 128], FP8, tag=f"vaug{a}")
                        nc.vector.memset(vaug[:, :, D:D + 1], 1.0)
                        vaugs.append(vaug)
                    for t0 in range(0, NT, 8):
                        pts = [
                            psum_av.tile(
                                [128, 8, D], FP32R, tag="av", name=f"pt{a}",
                                bufs=1,
                            )
                            for a in range(2)
                        ]
                        for g in range(8):
                            t = t0 + g
                            for a in range(2):
                                lo = a * D
                                nc.tensor.transpose(
                                    pts[a][:, g, :],
                                    self.vTt[lo:lo + D,
                                             t * 128:(t + 1) * 128],
                                    ident[lo:lo + D, lo:lo + D],
                                )
                        for a in range(2):
                            nc.vector.tensor_copy(
                                vaugs[a][:, t0:t0 + 8, 0:D], pts[a]
                            )
                            nc.vector.tensor_tensor(
                                vaugs[a][:, t0:t0 + 8, D + 1:128],
                                pts[a][:, :, 1:D],
                                vaugs[a][:, t0:t0 + 8, 1:D],
                                SUB,
                            )
                    return self.qTt, self.kTt, vaugs

            def emit_attention(hp, qTt, kTt, vaugs, qkv_cb=None, post_j=None):
                # ---- attention per q-chunk. aT tiles hold the full 128-row
                # [hi | denom*r | lo] block; proj contracts the doubled
                # channel dim with host-duplicated w_proj rows.
                aTts = [
                    hpool.tile([128, N], FP32R, tag=f"aTt{a}", name=f"aTt{a}")
                    for a in range(2)
                ]
                for j in range(NCH):
                    js = slice(j * NQ, (j + 1) * NQ)
                    expSs = [
                        spool.tile(
                            [128, NT, NQ], FP8,
                            tag=f"expS{a}", name=f"expS{a}", bufs=2,
                        )
                        for a in range(2)
                    ]
                    def s_mm(out_ap, a, t):
                        lo = a * D
                        kap = kTt[lo:lo + D, t * 128:(t + 1) * 128]
                        nc.tensor.matmul(
                            out_ap,
                            _dup2(kap),
                            qTt[lo:lo + D, :, js],
                            start=True,
                            stop=True,
                            perf_mode=DR,
                        )

                    def emit_act_group(g, heads=(0, 1)):
                        # tiles (2g, 2g+1) -> ScalarE exp, own sps0/1 ring
                        t = 2 * g
                        sps_ab = {
                            a: psum_s.tile(
                                [128, 2, NQ], FP32, tag=f"sps{a}",
                                name=f"sps{a}", bufs=1,
                            )
                            for a in heads
                        }
                        for a in reversed(heads):
                            for u in range(2):
                                s_mm(sps_ab[a][:, u, :], a, t + u)
                        for a in reversed(heads):
                            nc.scalar.activation(
                                out=expSs[a][:, t:t + 2, :],
                                in_=sps_ab[a][:, :, :],
                                func=EXP,
                                scale=SCALE,
                            )

                    def emit_schr_tile(t, heads=(0, 1)):
                        # kv-tile on the DVE schraudolph ring. When both
                        # heads run, their S psums share one 2-bank spsX
                        # tile so the DVE affine is a single [128,2,512] op.
                        spsx = psum_s.tile(
                            [128, 2, NQ], FP32, tag="spsX",
                            name="spsX", bufs=1,
                        )
                        for i, a in enumerate(heads):
                            s_mm(spsx[:, i, :], a, t)
                        i16 = small.tile(
                            [128, 2, NQ], I16, tag="i16", name="i16",
                        )
                        n_h = len(heads)
                        nc.vector.tensor_scalar(
                            i16[:, 0:n_h, :], spsx[:, 0:n_h, :],
                            SCHR_A, SCHR_B, MULT, ADD,
                        )
                        for i, a in enumerate(heads):
                            nc.gpsimd.tensor_copy(
                                expSs[a][:, t:t + 1, :],
                                i16[:, i, :].bitcast(BF16),
                            )

                    def emit_act_single(t, a):
                        # one kv-tile, one head on ScalarE
                        sps = psum_s.tile(
                            [128, 2, NQ], FP32, tag=f"sps{a}",
                            name=f"sps{a}", bufs=1,
                        )
                        s_mm(sps[:, 0, :], a, t)
                        nc.scalar.activation(
                            out=expSs[a][:, t:t + 1, :],
                            in_=sps[:, 0:1, :],
                            func=EXP,
                            scale=SCALE,
                        )

                    # head0: act tiles 0-9 + single 10, schr 11-15;
                    # head1: act tiles 0-11, schr 12-15. Tiles 12-15 run
                    # both heads in one paired DVE affine; the asymmetric
                    # split moves half a tile of exp off the ScalarE pacer.
                    emit_act_group(0)
                    emit_schr_tile(12)
                    emit_act_group(1)
                    emit_schr_tile(13)
                    emit_act_group(2)
                    odd = (hp + j) % 2 == 1
                    if odd:
                        emit_schr_tile(11, heads=(0,))
                    emit_act_group(3)
                    emit_schr_tile(14)
                    emit_act_group(4)
                    emit_schr_tile(15)
                    if odd:
                        emit_act_group(5, heads=(1,))
                        emit_act_single(10, 0)
                    else:
                        emit_act_group(5)
                    avs = []
                    for a in range(2):
                        av = psum_av.tile([128, NQ], FP32, tag="av", bufs=1)
                        for u in range(NT // 2):
                            nc.tensor.matmul(
                                av,
                                vaugs[a][:, 2 * u:2 * u + 2, :],
                                expSs[a][:, 2 * u:2 * u + 2, :],
                                start=(u == 0),
                                stop=(u == NT // 2 - 1),
                                perf_mode=DR,
                            )
                        avs.append(av)
                    # next pair's qkv copies go ahead of the normalize in
                    # the DVE stream: they only wait on quick qkv matmuls,
                    # while the normalize waits on the whole AV chain
                    if qkv_cb is not None:
                        qkv_cb(j)
                    for a in range(2):
                        av = avs[a]
                        # normalize all 128 rows at once (partition count is
                        # free); row 64 becomes denom*recip = 1 and hits a
                        # zero row of the extended w_proj
                        recip = small.tile([1, NQ], FP32, tag="recip")
                        nc.vector.reciprocal(recip, av[D:D + 1, :])
                        bc128 = small.tile([128, NQ], FP32, tag="bc128")
                        nc.gpsimd.partition_broadcast(bc128, recip)
                        nc.vector.tensor_tensor(
                            aTts[a][:, js], av, bc128, MULT
                        )
                    # half-pair aT stores; the last pair skips them (its
                    # aT feeds the proj straight from SBUF)
                    if j % 2 == 1 and hp < H // 2 - 1:
                        js2 = slice((j - 1) * NQ, (j + 1) * NQ)
                        for a in range(2):
                            h = 2 * hp + a
                            nc.sync.dma_start(
                                out=aT_d[h * 128:(h + 1) * 128, js2],
                                in_=aTts[a][:, js2],
                            )
                    if post_j is not None:
                        post_j(j)
                return aTts

            # proj weights/bias DMA'd up front on the gpsimd DMA queue so
            # they don't delay the first pair's x/wq loads on SP
            bias_bc = small.tile([128, C], FP32, tag="bias", bufs=1)
            nc.gpsimd.dma_start(out=bias_bc, in_=_bc_ap(b_proj[:], 128))
            w_projT = small.tile([128, 2 * CT, C], FP32R, tag="wproj",
                                 bufs=1)
            nc.gpsimd.dma_start(
                out=w_projT, in_=wpr.rearrange("(ko p) o -> p ko o", p=128)
            )

            NO = 384

            def emit_proj(i2_list, psum_tags, aT5=None):
                for n_p, i2 in enumerate(i2_list):
                    a_sb = small.tile(
                        [128, 2 * CT, 256], FP32R, tag="a_sb", bufs=2
                    )
                    nc.sync.dma_start(
                        out=a_sb,
                        in_=aT_d[:, i2 * 256:(i2 + 1) * 256].rearrange(
                            "(ko p) n -> p ko n", p=128
                        ),
                    )
                    for ii in range(2):
                        i = 2 * i2 + ii
                        y_sb = small.tile([128, C], FP32, tag="y_sb", bufs=2)
                        for half in range(2):
                            tg = psum_tags[(n_p * 4 + ii * 2 + half)
                                           % len(psum_tags)]
                            if tg == "qkvps":
                                psf = psum_qkv.tile(
                                    [128, NQ], FP32, tag="qkvps",
                                    name="qkvps")
                            elif tg == "spsX":
                                psf = psum_s.tile(
                                    [128, NQ], FP32, tag="spsX",
                                    name="spsX", bufs=1)
                            elif tg in ("sps0", "sps1"):
                                psf = psum_s.tile(
                                    [128, 2, NQ], FP32, tag=tg,
                                    name=tg, bufs=1)[:, 0, :]
                            else:
                                psf = psum_av.tile(
                                    [128, NQ], FP32, tag="av", bufs=1)
                            ps = psf[:, 0:NO]
                            isl = slice(i2 * 256 + ii * 128,
                                        i2 * 256 + (ii + 1) * 128)
                            for k in range(2 * CT):
                                # last pair's chunks come straight from its
                                # SBUF aT tiles - no DRAM roundtrip wait
                                lhs = (a_sb[:, k, ii * 128:(ii + 1) * 128]
                                       if k < 10 else aT5[k - 10][:, isl])
                                nc.tensor.matmul(
                                    ps,
                                    lhs,
                                    w_projT[:, k, half * NO:(half + 1) * NO],
                                    start=(k == 0),
                                    stop=(k == 2 * CT - 1),
                                )
                            nc.vector.tensor_add(
                                y_sb[:, half * NO:(half + 1) * NO], ps,
                                bias_bc[:, half * NO:(half + 1) * NO]
                            )
                        nc.sync.dma_start(
                            out=y[i * 128:(i + 1) * 128, :],
                            in_=y_sb,
                        )

            def last_pair_post_j(j):
                # overlap most of the projection with the last pair's
                # attention, using the qkv psum bank plus the freed sps1 ring
                pass

            cur = QkvPhase(0, rot=("qkvps", "spsX"))
            for j in range(NCH):
                cur.emit_j(j)
            tiles = cur.finish()
            aT5 = None
            for hp in range(H // 2):
                nxt = QkvPhase(hp + 1) if hp + 1 < H // 2 else None
                ret = emit_attention(
                    hp, *tiles,
                    qkv_cb=nxt.emit_j if nxt is not None else None,
                    post_j=last_pair_post_j if hp == H // 2 - 1 else None,
                )
                if hp == H // 2 - 1:
                    aT5 = ret
                if nxt is not None:
                    tiles = nxt.finish()

            # ---------- rest of the projection; attention psums now free
            emit_proj([0, 1, 2, 3, 4, 5, 6, 7],
                      ["qkvps", "sps0", "sps1", "spsX", "av"], aT5=aT5)
        const_cm.__exit__(None, None, None)
    lp.__exit__(None, None, None)

    nc.finalize()
    return nc


def get_nc():
    global _CACHED_NC
    if _CACHED_NC is None:
        _CACHED_NC = build()
    return _CACHED_NC


LAST_RESULT = None


def kernel(x, w_qkv, w_proj, b_proj, **run_kwargs):
    x = np.ascontiguousarray(np.asarray(x, dtype=np.float32))
    w_qkv = np.ascontiguousarray(np.asarray(w_qkv, dtype=np.float32))
    w_proj = np.ascontiguousarray(np.asarray(w_proj, dtype=np.float32))
    b_proj = np.ascontiguousarray(np.asarray(b_proj, dtype=np.float32))
    assert x.shape == (B, N, C)

    nc = get_nc()
    w_qkvT = np.ascontiguousarray(w_qkv.T)
    # extended proj weights: per head the aT block is
    # [hi d0-63 | denom*r (==1) | v_lo d1-63]; duplicate w rows for the lo
    # part, zero the denom row.
    w_ext = np.zeros((2 * C, C), dtype=np.float32)
    for h in range(H):
        blk = w_proj[:, h * D:(h + 1) * D]          # [C_out, 64]
        w_ext[h * 128:h * 128 + 64, :] = blk.T
        w_ext[h * 128 + 65:h * 128 + 128, :] = blk.T[1:64]
    w_projT = np.ascontiguousarray(w_ext)
    b_eff = np.ascontiguousarray(b_proj.astype(np.float32))
    in_maps = [
        {
            "xT": np.ascontiguousarray(x[i].T),
            "w_qkvT": w_qkvT,
            "w_projT": w_projT,
            "b_proj": b_eff,
        }
        for i in range(B)
    ]
    res = run_bass_kernel_spmd(nc, in_maps, list(range(B)), **run_kwargs)
    global LAST_RESULT
    LAST_RESULT = res
    out = np.stack([res.results[i]["y"] for i in range(B)], axis=0)
    return out


if __name__ == "__main__":
    rng = np.random.default_rng(0)
    x = rng.standard_normal((B, N, C), dtype=np.float32)
    w_qkv = (rng.standard_normal((F, C)) * 0.02).astype(np.float32)
    w_proj = (rng.standard_normal((C, C)) * 0.02).astype(np.float32)
    b_proj = (rng.standard_normal((C,)) * 0.02).astype(np.float32)
    out = kernel(x=x, w_qkv=w_qkv, w_proj=w_proj, b_proj=b_proj)
    print("out", out.shape, out.dtype, float(np.abs(out).max()))
